# revision 3
# baseline (speedup 1.0000x reference)
"""Trainium2 Bass kernel for nn_AttentionBlock (B=4, H=W=64, C=256, D=32).

Sharding: 8 shards = 4 samples x 2 query-halves. Each core gets the full
sample's rows (reordered so its 2048 query rows come first), computes K/V
for all 4096 keys, and attention + output projection + residual for its
2048 queries. No collectives needed.

Self-contained: hardcodes shapes, imports only /opt/trn_rl_repo concourse.
"""

import sys

if "/opt/trn_rl_repo" not in sys.path:
    sys.path.insert(0, "/opt/trn_rl_repo")

import numpy as np
import ml_dtypes

BF16 = ml_dtypes.bfloat16

# Problem constants
B, HH, WW, C = 4, 64, 64, 256
D = 32
N = HH * WW          # 4096 keys per sample
NQ = N // 2          # 2048 queries per core
NCORES = 8
KC = N // 128        # 32 key chunks
QB = NQ // 128       # 16 query blocks per core

_compiled_cache = {}


def _build(use_bias: bool):
    from contextlib import ExitStack
    from concourse import bacc, tile, mybir, masks

    f32 = mybir.dt.float32
    bf = mybir.dt.bfloat16

    nc = bacc.Bacc("TRN2", target_bir_lowering=False, debug=False, num_devices=NCORES)

    x16_d = nc.dram_tensor("x16", [N, C], bf, kind="ExternalInput")
    xq32_d = nc.dram_tensor("xq32", [NQ, C], f32, kind="ExternalInput")
    wqa_d = nc.dram_tensor("wqa_rep", [257, 128], bf, kind="ExternalInput")
    wka_d = nc.dram_tensor("wka_rep", [257, 128], bf, kind="ExternalInput")
    wva_d = nc.dram_tensor("wva", [257, 256], bf, kind="ExternalInput")
    woa_d = nc.dram_tensor("woa", [257, 256], bf, kind="ExternalInput")
    out_d = nc.dram_tensor("out", [NQ, C], f32, kind="ExternalOutput")

    Exp = mybir.ActivationFunctionType.Exp
    Add = mybir.AluOpType.add
    Mult = mybir.AluOpType.mult

    with tile.TileContext(nc) as tc:
        with ExitStack() as ctx:
            const = ctx.enter_context(tc.tile_pool(name="const", bufs=1))
            big = ctx.enter_context(tc.tile_pool(name="big", bufs=1))
            x16p = ctx.enter_context(tc.tile_pool(name="x16p", bufs=4))
            expp = ctx.enter_context(tc.tile_pool(name="expp", bufs=8))
            small = ctx.enter_context(tc.tile_pool(name="small", bufs=2))
            ps_s = ctx.enter_context(tc.tile_pool(name="ps_s", bufs=4, space="PSUM"))
            ps_att = ctx.enter_context(tc.tile_pool(name="ps_att", bufs=2, space="PSUM"))
            ps_misc = ctx.enter_context(tc.tile_pool(name="ps_misc", bufs=2, space="PSUM"))

            # ---- constants & weights ----
            ident = const.tile([128, 128], bf, tag="ident")
            masks.make_identity(nc, ident[:])
            ones_row = const.tile([1, 512], bf, tag="ones_row")
            nc.gpsimd.memset(ones_row[:], 1.0)

            wq0 = const.tile([128, 128], bf, tag="wq0")
            wq1 = const.tile([128, 128], bf, tag="wq1")
            wk0 = const.tile([128, 128], bf, tag="wk0")
            wk1 = const.tile([128, 128], bf, tag="wk1")
            wv0 = const.tile([128, 256], bf, tag="wv0")
            wv1 = const.tile([128, 256], bf, tag="wv1")
            wo0 = const.tile([128, 256], bf, tag="wo0")
            wo1 = const.tile([128, 256], bf, tag="wo1")
            nc.sync.dma_start(out=wq0[:], in_=wqa_d[0:128, :])
            nc.sync.dma_start(out=wq1[:], in_=wqa_d[128:256, :])
            nc.sync.dma_start(out=wk0[:], in_=wka_d[0:128, :])
            nc.sync.dma_start(out=wk1[:], in_=wka_d[128:256, :])
            nc.sync.dma_start(out=wv0[:], in_=wva_d[0:128, :])
            nc.sync.dma_start(out=wv1[:], in_=wva_d[128:256, :])
            nc.sync.dma_start(out=wo0[:], in_=woa_d[0:128, :])
            nc.sync.dma_start(out=wo1[:], in_=woa_d[128:256, :])
            if use_bias:
                wqb = const.tile([1, 128], bf, tag="wqb")
                wkb = const.tile([1, 128], bf, tag="wkb")
                wvb = const.tile([1, 256], bf, tag="wvb")
                wob = const.tile([1, 256], bf, tag="wob")
                nc.sync.dma_start(out=wqb[:], in_=wqa_d[256:257, :])
                nc.sync.dma_start(out=wkb[:], in_=wka_d[256:257, :])
                nc.sync.dma_start(out=wvb[:], in_=wva_d[256:257, :])
                nc.sync.dma_start(out=wob[:], in_=woa_d[256:257, :])

            # ---- phase A: x -> xT (channel-major), via identity matmuls ----
            xT = big.tile([128, 2, N], bf, tag="xT")  # [:, h, :]: channels 128h..128h+127
            for t in range(16):
                xa = x16p.tile([128, 256], bf, tag="x16")
                xb = x16p.tile([128, 256], bf, tag="x16")
                nc.sync.dma_start(out=xa[:], in_=x16_d[256 * t : 256 * t + 128, :])
                nc.sync.dma_start(out=xb[:], in_=x16_d[256 * t + 128 : 256 * t + 256, :])
                pt = ps_s.tile([128, 512], f32, tag="s")
                nc.tensor.matmul(pt[:, 0:128], xa[:, 0:128], ident[:], start=True, stop=True)
                nc.tensor.matmul(pt[:, 128:256], xb[:, 0:128], ident[:], start=True, stop=True)
                nc.tensor.matmul(pt[:, 256:384], xa[:, 128:256], ident[:], start=True, stop=True)
                nc.tensor.matmul(pt[:, 384:512], xb[:, 128:256], ident[:], start=True, stop=True)
                nc.any.tensor_copy(xT[:, 0, 256 * t : 256 * t + 256], pt[:, 0:256])
                nc.any.tensor_copy(xT[:, 1, 256 * t : 256 * t + 256], pt[:, 256:512])

            # ---- phase B: qT/kT (4x row-replicated) and V ----
            qT = big.tile([128, NQ], bf, tag="qT")
            kT = big.tile([128, N], bf, tag="kT")
            for s in range(NQ // 512):
                pq = ps_s.tile([128, 512], f32, tag="s")
                nc.tensor.matmul(pq[:], wq0[:], xT[:, 0, 512 * s : 512 * s + 512], start=True, stop=False)
                nc.tensor.matmul(pq[:], wq1[:], xT[:, 1, 512 * s : 512 * s + 512], start=False, stop=not use_bias)
                if use_bias:
                    nc.tensor.matmul(pq[:], wqb[:], ones_row[:, 0:512], start=False, stop=True)
                nc.any.tensor_copy(qT[:, 512 * s : 512 * s + 512], pq[:])
            for s in range(N // 512):
                pk = ps_s.tile([128, 512], f32, tag="s")
                nc.tensor.matmul(pk[:], wk0[:], xT[:, 0, 512 * s : 512 * s + 512], start=True, stop=False)
                nc.tensor.matmul(pk[:], wk1[:], xT[:, 1, 512 * s : 512 * s + 512], start=False, stop=not use_bias)
                if use_bias:
                    nc.tensor.matmul(pk[:], wkb[:], ones_row[:, 0:512], start=False, stop=True)
                nc.any.tensor_copy(kT[:, 512 * s : 512 * s + 512], pk[:])

            # V rows (keys) with a ones column at 256 for the softmax denominator
            vsb = big.tile([128, KC, 260], bf, tag="vsb")
            nc.vector.memset(vsb[:, :, 256:257], 1.0)
            for m in range(KC):
                pv = ps_misc.tile([128, 256], f32, tag="m")
                nc.tensor.matmul(pv[:], xT[:, 0, 128 * m : 128 * m + 128], wv0[:], start=True, stop=False)
                nc.tensor.matmul(pv[:], xT[:, 1, 128 * m : 128 * m + 128], wv1[:], start=False, stop=not use_bias)
                if use_bias:
                    nc.tensor.matmul(pv[:], ones_row[:, 0:128], wvb[:], start=False, stop=True)
                nc.any.tensor_copy(vsb[:, m, 0:256], pv[:])

            # ---- phase C: software-pipelined S -> exp -> attend, per 128-query block ----
            def epilogue(qb, pa):
                rec = small.tile([128, 1], f32, tag="rec")
                nc.vector.reciprocal(rec[:], pa[:, 256:257])
                at = small.tile([128, 256], bf, tag="attn")
                nc.vector.tensor_scalar(at[:], pa[:, 0:256], rec[:], None, Mult)
                ptr = ps_misc.tile([128, 256], f32, tag="m")
                nc.tensor.matmul(ptr[:, 0:128], at[:, 0:128], ident[:], start=True, stop=True)
                nc.tensor.matmul(ptr[:, 128:256], at[:, 128:256], ident[:], start=True, stop=True)
                aT = small.tile([128, 256], bf, tag="aT")
                nc.any.tensor_copy(aT[:], ptr[:])
                po = ps_misc.tile([128, 256], f32, tag="m")
                nc.tensor.matmul(po[:], aT[:, 0:128], wo0[:], start=True, stop=False)
                nc.tensor.matmul(po[:], aT[:, 128:256], wo1[:], start=False, stop=not use_bias)
                if use_bias:
                    nc.tensor.matmul(po[:], ones_row[:, 0:128], wob[:], start=False, stop=True)
                xq = small.tile([128, 256], f32, tag="xq", bufs=3)
                nc.sync.dma_start(out=xq[:], in_=xq32_d[128 * qb : 128 * qb + 128, :])
                ot = small.tile([128, 256], f32, tag="ot", bufs=3)
                nc.vector.tensor_tensor(ot[:], po[:], xq[:], Add)
                nc.sync.dma_start(out=out_d[128 * qb : 128 * qb + 128, :], in_=ot[:])

            pa_tiles = {}
            prev = None  # (ets, qb, G)
            for s in range(2 * QB + 1):
                if s < 2 * QB:
                    qb, G = divmod(s, 2)
                    if G == 0:
                        pa_tiles[qb] = ps_att.tile([128, 260], f32, tag="a", name=f"pa{qb}")
                    # scores S^T for 16 key chunks, 4x row-packed
                    pss = []
                    for i in range(4):
                        ps_i = ps_s.tile([128, 512], f32, tag="s")
                        pss.append(ps_i)
                    for gg in range(4):
                        for i in range(4):
                            m = 16 * G + 4 * gg + i
                            nc.tensor.matmul(
                                pss[i][:, 128 * gg : 128 * gg + 128],
                                kT[32 * i : 32 * i + 32, 128 * m : 128 * m + 128],
                                qT[32 * i : 32 * i + 32, 128 * qb : 128 * qb + 128],
                                start=True,
                                stop=True,
                                tile_position=(32 * i, 0),
                            )
                # attend with previous step's exp tiles (keeps PE busy during exp)
                if prev is not None:
                    ets_p, qb_p, G_p = prev
                    pa_p = pa_tiles[qb_p]
                    for gg in range(4):
                        for i in range(4):
                            m = 16 * G_p + 4 * gg + i
                            nc.tensor.matmul(
                                pa_p[:, 0:257],
                                ets_p[i][:, 128 * gg : 128 * gg + 128],
                                vsb[:, m, 0:257],
                                start=(m == 0),
                                stop=(m == KC - 1),
                            )
                    if G_p == 1:
                        epilogue(qb_p, pa_p)
                        del pa_tiles[qb_p]
                if s < 2 * QB:
                    ets = []
                    for i in range(4):
                        et = expp.tile([128, 512], bf, tag="e")
                        nc.scalar.activation(et[:], pss[i][:], Exp)
                        ets.append(et)
                    prev = (ets, qb, G)

    nc.compile()
    return nc


def _get_compiled(use_bias: bool):
    key = bool(use_bias)
    if key not in _compiled_cache:
        _compiled_cache[key] = _build(use_bias)
    return _compiled_cache[key]


def _prep(x, wq, bq, wk, bk, wv, bv, wo, bo):
    xf = np.ascontiguousarray(np.asarray(x, dtype=np.float32)).reshape(B, N, C)
    wq = np.asarray(wq, np.float32)
    bq = np.asarray(bq, np.float32)
    wk = np.asarray(wk, np.float32)
    bk = np.asarray(bk, np.float32)
    wv = np.asarray(wv, np.float32)
    bv = np.asarray(bv, np.float32)
    wo = np.asarray(wo, np.float32)
    bo = np.asarray(bo, np.float32)

    use_bias = not (
        np.all(bq == 0) and np.all(bk == 0) and np.all(bv == 0) and np.all(bo == 0)
    )

    scale = np.float32(1.0 / np.sqrt(np.float32(D)))
    wqa = np.concatenate([wq, bq[None, :]], 0) * scale  # fold softmax scale into q
    wka = np.concatenate([wk, bk[None, :]], 0)
    wqa_rep = np.ascontiguousarray(np.tile(wqa, (1, 4))).astype(BF16)  # [257, 128]
    wka_rep = np.ascontiguousarray(np.tile(wka, (1, 4))).astype(BF16)
    wva = np.concatenate([wv, bv[None, :]], 0).astype(BF16)  # [257, 256]
    woa = np.concatenate([wo, bo[None, :]], 0).astype(BF16)

    in_maps = []
    for core in range(NCORES):
        b, h = divmod(core, 2)
        if h == 0:
            xo = xf[b]
        else:
            xo = np.concatenate([xf[b, NQ:], xf[b, :NQ]], 0)
        in_maps.append(
            {
                "x16": xo.astype(BF16),
                "xq32": np.ascontiguousarray(xo[:NQ]),
                "wqa_rep": wqa_rep,
                "wka_rep": wka_rep,
                "wva": wva,
                "woa": woa,
            }
        )
    return in_maps, use_bias


def _gather(results):
    out = np.empty((B, N, C), np.float32)
    for core in range(NCORES):
        b, h = divmod(core, 2)
        out[b, NQ * h : NQ * (h + 1)] = results[core]["out"]
    return out.reshape(B, HH, WW, C)


def kernel(x, wq, bq, wk, bk, wv, bv, wo, bo):
    from concourse.bass_utils import run_bass_kernel_spmd

    in_maps, use_bias = _prep(x, wq, bq, wk, bk, wv, bv, wo, bo)
    nc = _get_compiled(use_bias)
    res = run_bass_kernel_spmd(nc, in_maps, core_ids=list(range(NCORES)))
    return _gather(res.results)


def _ensure_ntff_hook():
    """The agent image's antenv stub lacks axon_hooks; synthesize it so
    run_bass_kernel_spmd(trace=True) can NTFF-profile via libaxon_pjrt."""
    import types

    try:
        from antenv.axon_hooks import get_axon_ntff_profile_hook  # noqa: F401
        return
    except ImportError:
        pass
    import antenv
    from trn_agent_boot.trn_boot import _ntff_profile_via_ctypes

    mod = types.ModuleType("antenv.axon_hooks")
    state = {"h": _ntff_profile_via_ctypes("/opt/axon/libaxon_pjrt.so")}
    mod.get_axon_ntff_profile_hook = lambda: state["h"]
    mod.set_axon_ntff_profile_hook = lambda h: state.__setitem__("h", h)
    sys.modules["antenv.axon_hooks"] = mod
    antenv.axon_hooks = mod


def run_traced(inputs, **kw):
    """For test.py: run with NTFF profiling; returns (output, BassKernelResults)."""
    from concourse.bass_utils import run_bass_kernel_spmd

    _ensure_ntff_hook()

    in_maps, use_bias = _prep(**inputs)
    nc = _get_compiled(use_bias)
    res = run_bass_kernel_spmd(nc, in_maps, core_ids=list(range(NCORES)), trace=True, **kw)
    return _gather(res.results), res


# revision 10
# speedup vs baseline: 1.1117x; 1.1117x over previous
"""Trainium2 Bass kernel for nn_AttentionBlock (B=4, H=W=64, C=256, D=32).

Sharding: 8 shards = 4 samples x 2 query-halves. Each core gets the full
sample's rows (reordered so its 2048 query rows come first), computes K/V
for all 4096 keys, and attention + output projection + residual for its
2048 queries. No collectives needed.

Self-contained: hardcodes shapes, imports only /opt/trn_rl_repo concourse.
"""

import sys

if "/opt/trn_rl_repo" not in sys.path:
    sys.path.insert(0, "/opt/trn_rl_repo")

import numpy as np
import ml_dtypes

BF16 = ml_dtypes.bfloat16

# Problem constants
B, HH, WW, C = 4, 64, 64, 256
D = 32
N = HH * WW          # 4096 keys per sample
NQ = N // 2          # 2048 queries per core
NCORES = 8
KC = N // 128        # 32 key chunks
QB = NQ // 128       # 16 query blocks per core

_compiled_cache = {}


def _build(use_bias: bool):
    from contextlib import ExitStack
    from concourse import bacc, tile, mybir, masks

    f32 = mybir.dt.float32
    bf = mybir.dt.bfloat16

    nc = bacc.Bacc("TRN2", target_bir_lowering=False, debug=False, num_devices=NCORES)

    x16_d = nc.dram_tensor("x16", [N, C], bf, kind="ExternalInput")
    xq32_d = nc.dram_tensor("xq32", [NQ, C], f32, kind="ExternalInput")
    wqa_d = nc.dram_tensor("wqa_rep", [257, 128], bf, kind="ExternalInput")
    wka_d = nc.dram_tensor("wka_rep", [257, 128], bf, kind="ExternalInput")
    wva_d = nc.dram_tensor("wva", [257, 256], bf, kind="ExternalInput")
    woa_d = nc.dram_tensor("woa", [257, 256], bf, kind="ExternalInput")
    out_d = nc.dram_tensor("out", [NQ, C], f32, kind="ExternalOutput")

    Exp = mybir.ActivationFunctionType.Exp
    Add = mybir.AluOpType.add
    Mult = mybir.AluOpType.mult

    with tile.TileContext(nc) as tc:
        with ExitStack() as ctx:
            const = ctx.enter_context(tc.tile_pool(name="const", bufs=1))
            big = ctx.enter_context(tc.tile_pool(name="big", bufs=1))
            x16p = ctx.enter_context(tc.tile_pool(name="x16p", bufs=4))
            expp = ctx.enter_context(tc.tile_pool(name="expp", bufs=6))
            small = ctx.enter_context(tc.tile_pool(name="small", bufs=2))
            ps_s = ctx.enter_context(tc.tile_pool(name="ps_s", bufs=2, space="PSUM"))
            ps_att = ctx.enter_context(tc.tile_pool(name="ps_att", bufs=2, space="PSUM"))
            ps_misc = ctx.enter_context(tc.tile_pool(name="ps_misc", bufs=2, space="PSUM"))

            # ---- constants & weights ----
            ident = const.tile([128, 128], bf, tag="ident")
            masks.make_identity(nc, ident[:])
            ones_row = const.tile([1, 512], bf, tag="ones_row")
            nc.gpsimd.memset(ones_row[:], 1.0)

            wq0 = const.tile([128, 128], bf, tag="wq0")
            wq1 = const.tile([128, 128], bf, tag="wq1")
            wk0 = const.tile([128, 128], bf, tag="wk0")
            wk1 = const.tile([128, 128], bf, tag="wk1")
            wv0 = const.tile([128, 256], bf, tag="wv0")
            wv1 = const.tile([128, 256], bf, tag="wv1")
            wo0 = const.tile([128, 256], bf, tag="wo0")
            wo1 = const.tile([128, 256], bf, tag="wo1")
            nc.sync.dma_start(out=wq0[:], in_=wqa_d[0:128, :])
            nc.sync.dma_start(out=wq1[:], in_=wqa_d[128:256, :])
            nc.sync.dma_start(out=wk0[:], in_=wka_d[0:128, :])
            nc.sync.dma_start(out=wk1[:], in_=wka_d[128:256, :])
            nc.sync.dma_start(out=wv0[:], in_=wva_d[0:128, :])
            nc.sync.dma_start(out=wv1[:], in_=wva_d[128:256, :])
            nc.sync.dma_start(out=wo0[:], in_=woa_d[0:128, :])
            nc.sync.dma_start(out=wo1[:], in_=woa_d[128:256, :])
            if use_bias:
                wqb = const.tile([1, 128], bf, tag="wqb")
                wkb = const.tile([1, 128], bf, tag="wkb")
                wvb = const.tile([1, 256], bf, tag="wvb")
                wob = const.tile([1, 256], bf, tag="wob")
                nc.sync.dma_start(out=wqb[:], in_=wqa_d[256:257, :])
                nc.sync.dma_start(out=wkb[:], in_=wka_d[256:257, :])
                nc.sync.dma_start(out=wvb[:], in_=wva_d[256:257, :])
                nc.sync.dma_start(out=wob[:], in_=woa_d[256:257, :])

            # ---- phase A: x -> xT (channel-major), via identity matmuls ----
            xT = big.tile([128, 2, N], bf, tag="xT")  # [:, h, :]: channels 128h..128h+127
            for t in range(16):
                xa = x16p.tile([128, 256], bf, tag="x16")
                xb = x16p.tile([128, 256], bf, tag="x16")
                nc.sync.dma_start(out=xa[:], in_=x16_d[256 * t : 256 * t + 128, :])
                nc.sync.dma_start(out=xb[:], in_=x16_d[256 * t + 128 : 256 * t + 256, :])
                pt = ps_s.tile([128, 1024], f32, tag="s")
                nc.tensor.matmul(pt[:, 0:128], xa[:, 0:128], ident[:], start=True, stop=True)
                nc.tensor.matmul(pt[:, 128:256], xb[:, 0:128], ident[:], start=True, stop=True)
                nc.tensor.matmul(pt[:, 512:640], xa[:, 128:256], ident[:], start=True, stop=True)
                nc.tensor.matmul(pt[:, 640:768], xb[:, 128:256], ident[:], start=True, stop=True)
                nc.vector.tensor_copy(xT[:, 0, 256 * t : 256 * t + 256], pt[:, 0:256])
                nc.vector.tensor_copy(xT[:, 1, 256 * t : 256 * t + 256], pt[:, 512:768])

            # ---- phase B: qT/kT (4x row-replicated) and V ----
            qT = big.tile([128, NQ], bf, tag="qT")
            kT = big.tile([128, N], bf, tag="kT")
            for s in range(NQ // 512):
                pq = ps_s.tile([128, 1024], f32, tag="s")
                nc.tensor.matmul(pq[:, 0:512], wq0[:], xT[:, 0, 512 * s : 512 * s + 512], start=True, stop=False)
                nc.tensor.matmul(pq[:, 0:512], wq1[:], xT[:, 1, 512 * s : 512 * s + 512], start=False, stop=not use_bias)
                if use_bias:
                    nc.tensor.matmul(pq[:, 0:512], wqb[:], ones_row[:, 0:512], start=False, stop=True)
                nc.vector.tensor_copy(qT[:, 512 * s : 512 * s + 512], pq[:, 0:512])
            for s in range(N // 512):
                pk = ps_s.tile([128, 1024], f32, tag="s")
                nc.tensor.matmul(pk[:, 0:512], wk0[:], xT[:, 0, 512 * s : 512 * s + 512], start=True, stop=False)
                nc.tensor.matmul(pk[:, 0:512], wk1[:], xT[:, 1, 512 * s : 512 * s + 512], start=False, stop=not use_bias)
                if use_bias:
                    nc.tensor.matmul(pk[:, 0:512], wkb[:], ones_row[:, 0:512], start=False, stop=True)
                nc.vector.tensor_copy(kT[:, 512 * s : 512 * s + 512], pk[:, 0:512])

            # V rows (keys) with a ones column at 256 for the softmax denominator
            vsb = big.tile([128, KC, 260], bf, tag="vsb")
            nc.vector.memset(vsb[:, :, 256:257], 1.0)
            for m in range(KC):
                pv = ps_misc.tile([128, 256], f32, tag="m")
                nc.tensor.matmul(pv[:], xT[:, 0, 128 * m : 128 * m + 128], wv0[:], start=True, stop=False)
                nc.tensor.matmul(pv[:], xT[:, 1, 128 * m : 128 * m + 128], wv1[:], start=False, stop=not use_bias)
                if use_bias:
                    nc.tensor.matmul(pv[:], ones_row[:, 0:128], wvb[:], start=False, stop=True)
                nc.vector.tensor_copy(vsb[:, m, 0:256], pv[:])

            # ---- phase C: software-pipelined S -> exp -> attend, per 128-query block ----
            def epilogue(qb, pa):
                rec = small.tile([128, 1], f32, tag="rec")
                nc.vector.reciprocal(rec[:], pa[:, 256:257])
                at = small.tile([128, 256], bf, tag="attn")
                nc.vector.tensor_scalar(at[:], pa[:, 0:256], rec[:], None, Mult)
                ptr = ps_misc.tile([128, 256], f32, tag="m")
                nc.tensor.matmul(ptr[:, 0:128], at[:, 0:128], ident[:], start=True, stop=True)
                nc.tensor.matmul(ptr[:, 128:256], at[:, 128:256], ident[:], start=True, stop=True)
                aT = small.tile([128, 256], bf, tag="aT")
                nc.vector.tensor_copy(aT[:], ptr[:])
                po = ps_misc.tile([128, 256], f32, tag="m")
                nc.tensor.matmul(po[:], aT[:, 0:128], wo0[:], start=True, stop=False)
                nc.tensor.matmul(po[:], aT[:, 128:256], wo1[:], start=False, stop=not use_bias)
                if use_bias:
                    nc.tensor.matmul(po[:], ones_row[:, 0:128], wob[:], start=False, stop=True)
                xq = small.tile([128, 256], f32, tag="xq", bufs=3)
                nc.sync.dma_start(out=xq[:], in_=xq32_d[128 * qb : 128 * qb + 128, :])
                ot = small.tile([128, 256], f32, tag="ot", bufs=3)
                nc.vector.tensor_tensor(ot[:], po[:], xq[:], Add)
                nc.sync.dma_start(out=out_d[128 * qb : 128 * qb + 128, :], in_=ot[:])

            # steps of 8 key chunks; S cols interleaved across the tile's 2 PSUM
            # banks so the 4 row-group-concurrent matmuls drain to 2 banks.
            col_of = [512 * (j % 2) + 128 * (j // 2) for j in range(8)]
            pa_tiles = {}
            prev = None  # (et, qb, t)
            for s in range(4 * QB + 1):
                if s < 4 * QB:
                    qb, t = divmod(s, 4)
                    if t == 0:
                        pa_tiles[qb] = ps_att.tile([128, 260], f32, tag="a", name=f"pa{qb}")
                    pst = ps_s.tile([128, 1024], f32, tag="s")
                    for j in range(8):
                        m = 8 * t + j
                        # 2 row groups only: at most 2 matmuls in flight, and
                        # consecutive ones drain to alternating PSUM banks —
                        # never two concurrent drains into the same bank.
                        i = j % 2
                        nc.tensor.matmul(
                            pst[:, col_of[j] : col_of[j] + 128],
                            kT[32 * i : 32 * i + 32, 128 * m : 128 * m + 128],
                            qT[32 * i : 32 * i + 32, 128 * qb : 128 * qb + 128],
                            start=True,
                            stop=True,
                            tile_position=(32 * i, 0),
                        )
                # attend with previous step's exp tile (keeps PE busy during exp)
                if prev is not None:
                    et_p, qb_p, t_p = prev
                    pa_p = pa_tiles[qb_p]
                    for j in range(8):
                        m = 8 * t_p + j
                        nc.tensor.matmul(
                            pa_p[:, 0:257],
                            et_p[:, col_of[j] : col_of[j] + 128],
                            vsb[:, m, 0:257],
                            start=(m == 0),
                            stop=(m == KC - 1),
                        )
                    if t_p == 3:
                        epilogue(qb_p, pa_p)
                        del pa_tiles[qb_p]
                if s < 4 * QB:
                    et = expp.tile([128, 1024], bf, tag="e")
                    nc.scalar.activation(et[:], pst[:], Exp)
                    prev = (et, qb, t)

    nc.compile()
    return nc


def _get_compiled(use_bias: bool):
    key = bool(use_bias)
    if key not in _compiled_cache:
        _compiled_cache[key] = _build(use_bias)
    return _compiled_cache[key]


def _prep(x, wq, bq, wk, bk, wv, bv, wo, bo):
    xf = np.ascontiguousarray(np.asarray(x, dtype=np.float32)).reshape(B, N, C)
    wq = np.asarray(wq, np.float32)
    bq = np.asarray(bq, np.float32)
    wk = np.asarray(wk, np.float32)
    bk = np.asarray(bk, np.float32)
    wv = np.asarray(wv, np.float32)
    bv = np.asarray(bv, np.float32)
    wo = np.asarray(wo, np.float32)
    bo = np.asarray(bo, np.float32)

    use_bias = not (
        np.all(bq == 0) and np.all(bk == 0) and np.all(bv == 0) and np.all(bo == 0)
    )

    scale = np.float32(1.0 / np.sqrt(np.float32(D)))
    wqa = np.concatenate([wq, bq[None, :]], 0) * scale  # fold softmax scale into q
    wka = np.concatenate([wk, bk[None, :]], 0)
    wqa_rep = np.ascontiguousarray(np.tile(wqa, (1, 4))).astype(BF16)  # [257, 128]
    wka_rep = np.ascontiguousarray(np.tile(wka, (1, 4))).astype(BF16)
    wva = np.concatenate([wv, bv[None, :]], 0).astype(BF16)  # [257, 256]
    woa = np.concatenate([wo, bo[None, :]], 0).astype(BF16)

    in_maps = []
    for core in range(NCORES):
        b, h = divmod(core, 2)
        if h == 0:
            xo = xf[b]
        else:
            xo = np.concatenate([xf[b, NQ:], xf[b, :NQ]], 0)
        in_maps.append(
            {
                "x16": xo.astype(BF16),
                "xq32": np.ascontiguousarray(xo[:NQ]),
                "wqa_rep": wqa_rep,
                "wka_rep": wka_rep,
                "wva": wva,
                "woa": woa,
            }
        )
    return in_maps, use_bias


def _gather(results):
    out = np.empty((B, N, C), np.float32)
    for core in range(NCORES):
        b, h = divmod(core, 2)
        out[b, NQ * h : NQ * (h + 1)] = results[core]["out"]
    return out.reshape(B, HH, WW, C)


def kernel(x, wq, bq, wk, bk, wv, bv, wo, bo):
    from concourse.bass_utils import run_bass_kernel_spmd

    in_maps, use_bias = _prep(x, wq, bq, wk, bk, wv, bv, wo, bo)
    nc = _get_compiled(use_bias)
    res = run_bass_kernel_spmd(nc, in_maps, core_ids=list(range(NCORES)))
    return _gather(res.results)


def _ensure_ntff_hook():
    """The agent image's antenv stub lacks axon_hooks; synthesize it so
    run_bass_kernel_spmd(trace=True) can NTFF-profile via libaxon_pjrt."""
    import types

    try:
        from antenv.axon_hooks import get_axon_ntff_profile_hook  # noqa: F401
        return
    except ImportError:
        pass
    import antenv
    from trn_agent_boot.trn_boot import _ntff_profile_via_ctypes

    mod = types.ModuleType("antenv.axon_hooks")
    state = {"h": _ntff_profile_via_ctypes("/opt/axon/libaxon_pjrt.so")}
    mod.get_axon_ntff_profile_hook = lambda: state["h"]
    mod.set_axon_ntff_profile_hook = lambda h: state.__setitem__("h", h)
    sys.modules["antenv.axon_hooks"] = mod
    antenv.axon_hooks = mod


def run_traced(inputs, **kw):
    """For test.py: run with NTFF profiling; returns (output, BassKernelResults)."""
    from concourse.bass_utils import run_bass_kernel_spmd

    _ensure_ntff_hook()

    in_maps, use_bias = _prep(**inputs)
    nc = _get_compiled(use_bias)
    res = run_bass_kernel_spmd(nc, in_maps, core_ids=list(range(NCORES)), trace=True, **kw)
    return _gather(res.results), res
